# revision 1
# baseline (speedup 1.0000x reference)
"""Trainium2 Bass kernel for nn_ClusteringLayer (vq_codebook, Student-t assignments).

Computes, for x in R^{N x D} and clusters c in R^{K x D}:
    d2[n,k] = ||x_n - c_k||^2
    q = 1 / (1 + d2)            (Student-t, alpha=1, power=(alpha+1)/2=1)
    out = q / q.sum(-1, keepdims=True)

Strategy (data-parallel over 8 NeuronCores, cluster table replicated):
  - host: shard x along N (65536 rows/core), transpose+cast each shard to
    bf16 X^T [D, Nsh] so the contraction dim lands on SBUF partitions.
  - device, per 1024-column block, with C^T stationary on the PE:
      psum[k, n] = (-2C)^T.T @ X^T  +  ones.T @ (X^T)^2     (8 matmuls of 512)
      u[k, n]    = Ln(psum + (1 + ||c||^2))   [ACT; per-partition bias; fp16]
      u^T        = one DMA-xbar transpose  -> [n, k] layout [128, 8, 64]
      q          = Exp(-u^T)                 [ACT; bf16]
      s          = row-sum over k (DVE reduce), out = q * (1/s) (DVE), store.
  - host: upcast bf16 -> f32, concat shards.
"""

import numpy as np
from contextlib import ExitStack

N, D, K = 524288, 256, 64
NCORES = 8
NSH = N // NCORES  # 65536 rows per core
BLK = 2048         # n-columns per block
# DMA-xbar transpose of [64, BLK] -> [128, BLK//128, 64]: which logical row
# ordering the xbar produces.  "tp" => row r = t*128 + p ; "pt" => r = p*tp + t
XBAR_ORDER = "tp"


def _patch_act_tables():
    """Make Ln and Exp resolve to the single set that contains both
    (natural_log_exp_and_others), so the kernel pays one ACT_TABLE_LOAD
    instead of alternating sets every block.  Only values are modified --
    set order (and hence act_func_set_id indices) is preserved."""
    import functools
    from concourse import hw_specs, bacc, mybir

    if getattr(hw_specs, "_act_tables_patched", False):
        return
    orig = hw_specs.get_activation_tables

    @functools.cache
    def patched(arch):
        t = dict(orig(arch))
        ln = mybir.ActivationFunctionType.Ln
        ex = mybir.ActivationFunctionType.Exp
        out = {}
        for name, funcs in t.items():
            if name != "natural_log_exp_and_others" and (ln in funcs or ex in funcs):
                funcs = funcs - {ln, ex}
            out[name] = funcs
        return out

    hw_specs.get_activation_tables = patched
    bacc.get_activation_tables = patched
    hw_specs._act_tables_patched = True


def _build(nsh=NSH, blk=BLK):
    import concourse.bacc as bacc
    import concourse.tile as tile
    from concourse import mybir

    _patch_act_tables()

    f32 = mybir.dt.float32
    bf16 = mybir.dt.bfloat16
    f16 = mybir.dt.float16
    nblk = nsh // blk
    tp = blk // 128

    nc = bacc.Bacc("TRN2", target_bir_lowering=False, debug=False)
    xt = nc.dram_tensor("xt", [D, nsh], bf16, kind="ExternalInput").ap()
    # clusters arrive host-duplicated to [2K, D] so per-partition bias/c2
    # cover both packed k-halves of the psum
    cl = nc.dram_tensor("clusters", [2 * K, D], f32, kind="ExternalInput").ap()
    qo = nc.dram_tensor("q", [nsh, K], bf16, kind="ExternalOutput").ap()

    with tile.TileContext(nc) as tc, ExitStack() as ctx:
        wp = ctx.enter_context(tc.tile_pool(name="w", bufs=1))
        io = ctx.enter_context(tc.tile_pool(name="io", bufs=6))
        sqp = ctx.enter_context(tc.tile_pool(name="sq", bufs=6))
        up = ctx.enter_context(tc.tile_pool(name="u", bufs=6))
        qp = ctx.enter_context(tc.tile_pool(name="qp", bufs=6))
        sp = ctx.enter_context(tc.tile_pool(name="sp", bufs=8))
        pp = ctx.enter_context(tc.tile_pool(name="ps", bufs=3, space="PSUM"))
        dp = ctx.enter_context(tc.tile_pool(name="dmy", bufs=2, space="PSUM"))

        # --- one-time cluster prep (replicated on every core) ---
        cl_sb = wp.tile([2 * K, D], f32, tag="cl")
        nc.sync.dma_start(cl_sb, cl)
        csq = wp.tile([2 * K, D], f32, tag="csq")
        nc.vector.tensor_mul(csq, cl_sb, cl_sb)
        c2 = wp.tile([2 * K, 1], f32, tag="c2")
        nc.vector.tensor_reduce(c2, csq, axis=mybir.AxisListType.X,
                                op=mybir.AluOpType.add)
        c2p1 = wp.tile([2 * K, 1], f32, tag="c2p1")
        nc.vector.tensor_scalar_add(c2p1, c2, 1.0)
        cn2 = wp.tile([K, D], bf16, tag="cn2")
        nc.vector.tensor_scalar_mul(cn2, cl_sb[0:K, :], -2.0)
        ct0 = wp.tile([128, K], bf16, tag="ct0")
        ct1 = wp.tile([128, K], bf16, tag="ct1")
        nc.sync.dma_start_transpose(ct0, cn2[:, 0:128])
        nc.sync.dma_start_transpose(ct1, cn2[:, 128:256])
        ones = wp.tile([128, K], bf16, tag="ones")
        nc.vector.memset(ones, 1.0)
        ones128 = wp.tile([128, 128], bf16, tag="ones128")
        nc.vector.memset(ones128, 1.0)

        def front(b):
            n0 = b * blk
            xt0 = io.tile([128, blk], bf16, tag="xt0")
            xt1 = io.tile([128, blk], bf16, tag="xt1")
            nc.sync.dma_start(xt0, xt[0:128, n0:n0 + blk])
            nc.sync.dma_start(xt1, xt[128:256, n0:n0 + blk])
            # squares for the x^2 row-norm term, spread over three engines
            xq0 = sqp.tile([128, blk], bf16, tag="xq0")
            xq1 = sqp.tile([128, blk], bf16, tag="xq1")
            nc.gpsimd.tensor_mul(xq0, xt0, xt0)
            nc.vector.tensor_mul(xq1, xt1, xt1)

            # packed psum [128, blk//2]: partitions 0:64 hold k for the first
            # blk//2 columns, 64:128 for the second half (col-tiled matmuls).
            half = blk // 2
            ps = pp.tile([128, half], f32, tag="d2")
            for g in range(2):
                rows = slice(g * K, (g + 1) * K)
                tpos = (0, g * K) if g else None
                for h in range(half // 512):
                    sl = slice(h * 512, h * 512 + 512)
                    xsl = slice(g * half + h * 512, g * half + h * 512 + 512)
                    nc.tensor.matmul(ps[rows, sl], ct0, xt0[:, xsl],
                                     start=True, stop=False, tile_position=tpos)
                    nc.tensor.matmul(ps[rows, sl], ct1, xt1[:, xsl],
                                     start=False, stop=False, tile_position=tpos)
            dmy = dp.tile([128, 64], f32, tag="d")
            nc.tensor.matmul(dmy, ones128, ones, start=True, stop=True)
            for g in range(2):
                rows = slice(g * K, (g + 1) * K)
                tpos = (0, g * K) if g else None
                for h in range(half // 512):
                    sl = slice(h * 512, h * 512 + 512)
                    xsl = slice(g * half + h * 512, g * half + h * 512 + 512)
                    nc.tensor.matmul(ps[rows, sl], ones, xq0[:, xsl],
                                     start=False, stop=False, tile_position=tpos)
                    nc.tensor.matmul(ps[rows, sl], ones, xq1[:, xsl],
                                     start=False, stop=True, tile_position=tpos)
            dmy2 = dp.tile([128, 64], f32, tag="d")
            nc.tensor.matmul(dmy2, ones128, ones, start=True, stop=True)

            # u = ln(1 + d2) ; bias adds (1 + ||c_k||^2) per partition
            u = up.tile([128, half], f16, tag="u")
            nc.scalar.activation(u, ps, func=mybir.ActivationFunctionType.Ln,
                                 bias=c2p1, scale=1.0)

            # one xbar transpose to [n, k] layout (Scalar HWDGE queue).
            # ut[p, s, g*K+k] = q-term for n = n0 + g*half + s*128 + p.
            ns = half // 128
            ut = up.tile([128, ns, 2 * K], f16, tag="ut")
            nc.scalar.dma_start_transpose(ut, u)
            return ut

        def back(b, ut):
            n0 = b * blk
            half = blk // 2
            ns = half // 128
            # q = exp(-u)
            qb = qp.tile([128, ns, 2 * K], bf16, tag="qb")
            nc.scalar.activation(qb, ut,
                                 func=mybir.ActivationFunctionType.Exp, scale=-1.0)

            qbv = qb.rearrange("p s (g k) -> p (s g) k", k=K)
            s = sp.tile([128, 2 * ns, 1], f32, tag="s")
            nc.vector.tensor_reduce(s, qbv, axis=mybir.AxisListType.X,
                                    op=mybir.AluOpType.add)
            sinv = sp.tile([128, 2 * ns, 1], f32, tag="sinv")
            nc.vector.reciprocal(sinv, s)
            sinvb = sp.tile([128, 2 * ns, 1], bf16, tag="sinvb")
            nc.vector.tensor_copy(sinvb, sinv)

            qn = qp.tile([128, 2 * ns, K], bf16, tag="qn")
            nc.vector.tensor_tensor(qn, qbv, sinvb.to_broadcast([128, 2 * ns, K]),
                                    op=mybir.AluOpType.mult)

            # row n0 + g*half + s*128 + p  <-  qn[p, (s g)]; one DMA per half
            qn4 = qn.rearrange("p (s g) k -> p s g k", g=2)
            for g in range(2):
                q_half = qo[n0 + g * half:n0 + (g + 1) * half, :].rearrange(
                    "(s p) k -> p s k", p=128)
                nc.sync.dma_start(q_half, qn4[:, :, g, :])

        for b in range(nblk):
            back(b, front(b))

    nc.compile()
    return nc


_CACHE = {}


def _get_nc(nsh=NSH, blk=BLK):
    key = (nsh, blk)
    if key not in _CACHE:
        _CACHE[key] = _build(nsh, blk)
    return _CACHE[key]


def kernel(inputs, clusters):
    import ml_dtypes
    from concourse.bass_utils import run_bass_kernel_spmd

    x = np.asarray(inputs)
    c = np.ascontiguousarray(np.asarray(clusters, dtype=np.float32))
    assert x.shape == (N, D) and c.shape == (K, D)

    nc = _get_nc()
    xb = x.astype(ml_dtypes.bfloat16)
    c2x = np.ascontiguousarray(np.concatenate([c, c], axis=0))  # [2K, D]
    in_maps = []
    for i in range(NCORES):
        xts = np.ascontiguousarray(xb[i * NSH:(i + 1) * NSH].T)  # [D, NSH] bf16
        in_maps.append({"xt": xts, "clusters": c2x})

    res = run_bass_kernel_spmd(nc, in_maps, core_ids=list(range(NCORES)))
    out = np.concatenate(
        [np.asarray(r["q"]).astype(np.float32) for r in res.results], axis=0
    )
    return out



# revision 9
# speedup vs baseline: 1.6405x; 1.6405x over previous
"""Trainium2 Bass kernel for nn_ClusteringLayer (vq_codebook, Student-t assignments).

Computes, for x in R^{N x D} and clusters c in R^{K x D}:
    d2[n,k] = ||x_n - c_k||^2
    q = 1 / (1 + d2)            (Student-t, alpha=1, power=(alpha+1)/2=1)
    out = q / q.sum(-1, keepdims=True)

Strategy (data-parallel over 8 NeuronCores, cluster table replicated):
  - host: shard x along N (65536 rows/core); each shard ships as an augmented
    transposed tile X~ [260, Nsh] bf16 whose rows are
        [ x^T (256) ; x2 - 256 ; 1 ; 1 ; 0 ]
    so the whole Student-t numerator 1 + d2 comes out of ONE matmul chain:
        psum[n,k] = sum_r X~[r,n] * M[r,k]
    with moving M = [ -2 c^T (256) ; 1 ; A ; B ; 0 ] where A + B is a
    two-term bf16 split of (257 + ||c_k||^2)  (so the bf16 rounding error of
    the large constant cancels to ~1e-2 absolute).
  - device, per 128-row n-tile, 3 matmuls with the X~ slice STATIONARY
    (psum lands in [n, k] layout directly -> no transpose anywhere):
        mm1: stat xt[0:128,t]   x mov ct0 [128,64]
        mm2: stat xt[128:256,t] x mov ct1 [128,64]
        mm3: stat xt[256:260,t] x mov ct2 [4,64]
    8 tiles accumulate into one PSUM bank [128, 8*64] as a single
    accumulation group (start on the first MM, stop on the last).
  - ACT: q = Reciprocal(psum) -> bf16 (one table set, loaded once)
  - GPSIMD: s = sum_k q ; DVE: sinv = 1/s, out = q * sinv -> bf16
  - out is stored in the SBUF-natural [p, s, k] layout (contiguous 4KB DMA
    lines); host inverts the (s p) interleave when unsharding.
"""

import numpy as np
from contextlib import ExitStack


def _patch_act_tables():
    """Make Ln and Exp resolve to the single set that contains both
    (natural_log_exp_and_others), so the kernel pays one ACT_TABLE_LOAD
    instead of alternating sets per instruction.  Only values are modified --
    set order (and hence act_func_set_id indices) is preserved."""
    import functools
    from concourse import hw_specs, bacc, mybir

    if getattr(hw_specs, "_act_tables_patched", False):
        return
    orig = hw_specs.get_activation_tables

    @functools.cache
    def patched(arch):
        t = dict(orig(arch))
        ln = mybir.ActivationFunctionType.Ln
        ex = mybir.ActivationFunctionType.Exp
        out = {}
        for name, funcs in t.items():
            if name != "natural_log_exp_and_others" and (ln in funcs or ex in funcs):
                funcs = funcs - {ln, ex}
            out[name] = funcs
        return out

    hw_specs.get_activation_tables = patched
    bacc.get_activation_tables = patched
    hw_specs._act_tables_patched = True

N, D, K = 524288, 256, 64
NCORES = 8
NSH = N // NCORES          # 65536 rows per core
XROWS = D + 4              # 256 data rows + [x2-256, 1, 1, 0]
BLK = 4096                 # n-columns per DMA block
CHUNK = 1024               # n-columns per PSUM bank (8 tiles of 128)
NT = NSH // 128            # 512 n-tiles per core


def _build(nsh=NSH, blk=BLK):
    import concourse.bacc as bacc
    import concourse.tile as tile
    from concourse import mybir

    _patch_act_tables()

    f32 = mybir.dt.float32
    bf16 = mybir.dt.bfloat16
    f16 = mybir.dt.float16
    nblk = nsh // blk
    tb = blk // 128            # n-tiles per block (32)
    nch = blk // CHUNK         # psum banks per block (4)
    ct = CHUNK // 128          # n-tiles per bank (8)

    nc = bacc.Bacc("TRN2", target_bir_lowering=False, debug=False)
    xt = nc.dram_tensor("xt", [XROWS, nsh], bf16, kind="ExternalInput").ap()
    cl = nc.dram_tensor("clusters", [K, D], f32, kind="ExternalInput").ap()
    # output in SBUF-natural layout: q_dev[p, s*K + k] = q[s*128 + p, k]
    qo = nc.dram_tensor("q", [128, (nsh // 128) * K], bf16,
                        kind="ExternalOutput").ap()

    with tile.TileContext(nc) as tc, ExitStack() as ctx:
        wp = ctx.enter_context(tc.tile_pool(name="w", bufs=1))
        io = ctx.enter_context(tc.tile_pool(name="io", bufs=3))
        qp = ctx.enter_context(tc.tile_pool(name="qp", bufs=2))
        up = ctx.enter_context(tc.tile_pool(name="up", bufs=3))
        sp = ctx.enter_context(tc.tile_pool(name="sp", bufs=3))
        op = ctx.enter_context(tc.tile_pool(name="op", bufs=2))
        pp = ctx.enter_context(tc.tile_pool(name="ps", bufs=4, space="PSUM"))

        # --- one-time cluster prep (replicated on every core) ---
        cl_sb = wp.tile([K, D], f32, tag="cl")
        nc.sync.dma_start(cl_sb, cl)
        csq = wp.tile([K, D], f32, tag="csq")
        nc.vector.tensor_mul(csq, cl_sb, cl_sb)
        c2 = wp.tile([K, 1], f32, tag="c2")
        nc.vector.tensor_reduce(c2, csq, axis=mybir.AxisListType.X,
                                op=mybir.AluOpType.add)
        t257 = wp.tile([K, 1], f32, tag="t257")
        nc.vector.tensor_scalar_add(t257, c2, 257.0)
        ab = wp.tile([K, 1], bf16, tag="ab")
        nc.vector.tensor_copy(ab, t257)
        a32 = wp.tile([K, 1], f32, tag="a32")
        nc.vector.tensor_copy(a32, ab)
        bres = wp.tile([K, 1], f32, tag="bres")
        nc.vector.tensor_sub(bres, t257, a32)
        bb = wp.tile([K, 1], bf16, tag="bb")
        nc.vector.tensor_copy(bb, bres)
        # colsrc columns [1, A, B, 0...] -> transpose -> ct2 rows
        colsrc = wp.tile([K, 128], bf16, tag="colsrc")
        nc.vector.memset(colsrc, 0.0)
        nc.vector.memset(colsrc[:, 0:1], 1.0)
        nc.vector.tensor_copy(colsrc[:, 1:2], ab)
        nc.vector.tensor_copy(colsrc[:, 2:3], bb)
        ct2 = wp.tile([128, K], bf16, tag="ct2")
        nc.sync.dma_start_transpose(ct2, colsrc)
        # ct0/ct1 = (-2 c)^T halves
        cn2 = wp.tile([K, D], bf16, tag="cn2")
        nc.vector.tensor_scalar_mul(cn2, cl_sb, -2.0)
        ct0 = wp.tile([128, K], bf16, tag="ct0")
        ct1 = wp.tile([128, K], bf16, tag="ct1")
        nc.sync.dma_start_transpose(ct0, cn2[:, 0:128])
        nc.sync.dma_start_transpose(ct1, cn2[:, 128:256])

        for b in range(nblk):
            n0 = b * blk
            xt0 = io.tile([128, blk], bf16, tag="xt0")
            xt1 = io.tile([128, blk], bf16, tag="xt1")
            xt2 = io.tile([4, blk], bf16, tag="xt2")
            nc.sync.dma_start(xt0, xt[0:128, n0:n0 + blk])
            nc.sync.dma_start(xt1, xt[128:256, n0:n0 + blk])
            nc.sync.dma_start(xt2, xt[256:260, n0:n0 + blk])

            qblk = qp.tile([128, tb, K], bf16, tag="qb")
            s = sp.tile([128, tb, 1], f32, tag="s")
            sinv = sp.tile([128, tb, 1], f32, tag="sinv")
            sinvb = sp.tile([128, tb, 1], bf16, tag="sinvb")
            outb = op.tile([128, tb, K], bf16, tag="outb")

            for c in range(nch):
                ps = pp.tile([128, ct * K], f32, tag="d2")
                for t in range(ct):
                    off = (c * ct + t) * 128
                    sl = slice(t * K, (t + 1) * K)
                    nc.tensor.matmul(ps[:, sl], xt0[:, off:off + 128], ct0,
                                     start=(t == 0), stop=False)
                    nc.tensor.matmul(ps[:, sl], xt1[:, off:off + 128], ct1,
                                     start=False, stop=False)
                    nc.tensor.matmul(ps[:, sl], xt2[:, off:off + 128],
                                     ct2[0:4, :], start=False,
                                     stop=(t == ct - 1))

                tsl = slice(c * ct, (c + 1) * ct)
                psv = ps.rearrange("p (t k) -> p t k", k=K)
                # q = exp(-ln(1+d2)) = 1/(1+d2); ACT Reciprocal is blocked
                # (HW accuracy), Ln+Exp share one table set.
                u = up.tile([128, ct, K], f16, tag="u")
                nc.scalar.activation(u, psv,
                                     func=mybir.ActivationFunctionType.Ln,
                                     scale=1.0)
                nc.scalar.activation(qblk[:, tsl, :], u,
                                     func=mybir.ActivationFunctionType.Exp,
                                     scale=-1.0)
                nc.vector.tensor_reduce(s[:, tsl, :], qblk[:, tsl, :],
                                        axis=mybir.AxisListType.X,
                                        op=mybir.AluOpType.add)
                nc.vector.reciprocal(sinv[:, tsl, :], s[:, tsl, :])
                nc.vector.tensor_copy(sinvb[:, tsl, :], sinv[:, tsl, :])
                nc.vector.tensor_tensor(
                    outb[:, tsl, :], qblk[:, tsl, :],
                    sinvb[:, tsl, :].to_broadcast([128, ct, K]),
                    op=mybir.AluOpType.mult)

            ov = outb.rearrange("p t k -> p (t k)")
            nc.scalar.dma_start(qo[:, b * tb * K:(b + 1) * tb * K], ov)

    nc.compile()
    return nc


_CACHE = {}


def _get_nc(nsh=NSH, blk=BLK):
    key = (nsh, blk)
    if key not in _CACHE:
        _CACHE[key] = _build(nsh, blk)
    return _CACHE[key]


def prep_in_maps(inputs, clusters):
    """Host-side shard/layout prep shared by kernel() and test harnesses."""
    import ml_dtypes

    x = np.asarray(inputs, dtype=np.float32)
    c = np.ascontiguousarray(np.asarray(clusters, dtype=np.float32))
    assert x.shape == (N, D) and c.shape == (K, D)

    in_maps = []
    for i in range(NCORES):
        xs = x[i * NSH:(i + 1) * NSH]
        aug = np.empty((XROWS, NSH), dtype=ml_dtypes.bfloat16)
        aug[0:D] = xs.T
        aug[D] = (xs * xs).sum(axis=1) - 256.0
        aug[D + 1] = 1.0
        aug[D + 2] = 1.0
        aug[D + 3] = 0.0
        in_maps.append({"xt": np.ascontiguousarray(aug), "clusters": c})
    return in_maps


def unshard(results):
    """[128, (NSH//128)*K] bf16 per core -> [N, K] f32."""
    outs = []
    for r in results:
        qd = np.asarray(r["q"]).reshape(128, NSH // 128, K)
        outs.append(qd.transpose(1, 0, 2).reshape(NSH, K).astype(np.float32))
    return np.concatenate(outs, axis=0)


def kernel(inputs, clusters):
    from concourse.bass_utils import run_bass_kernel_spmd

    nc = _get_nc()
    in_maps = prep_in_maps(inputs, clusters)
    res = run_bass_kernel_spmd(nc, in_maps, core_ids=list(range(NCORES)))
    return unshard(res.results)
